# revision 26
# baseline (speedup 1.0000x reference)
"""Trainium2 Bass kernel for nn_KeypointsLoss.

Math (per batch b):
    x[p,k] = trunc(kp[b,p,k,0] * (W-1)); y likewise from kp[...,1]
    g_row[p,k,h] = exp(-(h-x)^2/(2s^2)) * (vis>0);  g_col[p,k,w] = exp(-(w-y)^2/(2s^2))
    target[k] = sum_p outer(g_row, g_col)            # [H,W]
    per_sample = sum_k |pred[b,k] - target[k]|^2
    loss = sum_b per_sample / (sum(vis[b]) + 1e-6) / B

Strategy (8 cores, data-parallel over B=32 -> 4 batches/core):
  - pred streams in f32 over HWDGE (sync queue) as 8 [96, K*W] h-major
    tiles; kp/vis/k16 inputs ride the scalar HWDGE queue so nothing
    queues behind the bulk stream.
  - gcol is generated (ScalarE Square+Exp) directly into one [128, 3072]
    block-diagonal tile via strided-partition APs (partitions 32g+8j+p,
    fixed j per op) -- no staircase DMAs, no per-k copies.
  - Splat: per (b, g, half) two 384-col matmuls with tile_position=
    (32g, 0); the four 32-row groups run concurrently in the PE array.
  - Drain: DVE subtracts pred from the PSUM target (bf16 out), ScalarE
    squares+accumulates; 36 small units pipeline against the DMA stream.
  - Tiny matmul with a (1/32)-vector reduces partitions; visibility
    normalizer on-device; host sums the 8x4 partials.
"""

import sys
import numpy as np

sys.path.insert(0, "/opt/trn_rl_repo")

B, P, K, H, W = 32, 8, 17, 192, 192
SIGMA = 3.0
INV2S2 = 1.0 / (2.0 * SIGMA**2)
NCORES = 8
NB = B // NCORES          # batches per core
HL = 96                   # h split: [0:96) lo, [96:192) hi
KW = K * W                # 3264 free cols for pred tiles
NG = 4                    # full k-groups of 4 (k0..15); k=16 handled separately

_CACHE = {}


def _build():
    import concourse.bass as bass
    import concourse.bacc as bacc
    import concourse.tile as tile
    from concourse import mybir

    f32 = mybir.dt.float32
    bf16 = mybir.dt.bfloat16
    i32 = mybir.dt.int32
    Alu = mybir.AluOpType
    Act = mybir.ActivationFunctionType

    nc = bacc.Bacc("TRN2", target_bir_lowering=False, debug=False,
                   num_devices=NCORES)

    pred_d = nc.dram_tensor("pred", [NB, K, H, W], f32, kind="ExternalInput").ap()
    kp_d = nc.dram_tensor("kp", [NB, K, P, 2], f32, kind="ExternalInput").ap()
    vis_d = nc.dram_tensor("vis", [NB, K, P], i32, kind="ExternalInput").ap()
    iota_d = nc.dram_tensor("iota", [128, W], f32, kind="ExternalInput").ap()
    negi_d = nc.dram_tensor("negi", [HL, HL], bf16, kind="ExternalInput").ap()
    out_d = nc.dram_tensor("out", [NB, 1], f32, kind="ExternalOutput").ap()

    with tile.TileContext(nc) as tc:
        import contextlib
        with contextlib.ExitStack() as ctx:
            consts = ctx.enter_context(tc.tile_pool(name="consts", bufs=1))
            colp = ctx.enter_context(tc.tile_pool(name="cols", bufs=1))
            predp = ctx.enter_context(tc.tile_pool(name="pred", bufs=1))
            scrp = ctx.enter_context(tc.tile_pool(name="scr", bufs=4))
            sqp = ctx.enter_context(tc.tile_pool(name="sq", bufs=2))
            psump = ctx.enter_context(tc.tile_pool(name="psum", bufs=4, space="PSUM"))

            # ---- bulk pred stream: 8 f32 DMAs on the sync HWDGE queue.
            # kp/vis ride first (they gate all generation work).
            kpd = colp.tile([128, 2 * NB], f32, tag="kpd", name="kpd")
            visd = colp.tile([128, NB], i32, tag="visd", name="visd")
            nc.sync.dma_start(
                out=kpd[:].rearrange("p (b t) -> p b t", t=2),
                in_=kp_d.rearrange("b k p t -> (k p) b t")[0:128])
            nc.sync.dma_start(
                out=visd[:],
                in_=vis_d.rearrange("b k p -> (k p) b")[0:128])

            # pred streams via SWDGE with f32->bf16 cast (HWDGE chokes on the
            # 1632-descriptor pattern; Q7 generation keeps up and halves the
            # SBUF write traffic)
            plo_t, phi_t = [], []
            for b in range(NB):
                psrc = pred_d[b].rearrange("k h w -> h k w")
                plo = predp.tile([HL, KW], bf16, tag=f"plo{b}", name=f"plo{b}")
                phi = predp.tile([HL, KW], bf16, tag=f"phi{b}", name=f"phi{b}")
                nc.gpsimd.dma_start(
                    out=plo[:].rearrange("p (k w) -> p k w", w=W),
                    in_=psrc[0:HL])
                nc.gpsimd.dma_start(
                    out=phi[:].rearrange("p (k w) -> p k w", w=W),
                    in_=psrc[HL:H])
                plo_t.append(plo)
                phi_t.append(phi)

            # ---- small inputs on the scalar HWDGE queue
            iota_t = consts.tile([128, W], f32, tag="iota")
            kpt1 = colp.tile([128, 2], f32, tag="kpt1", name="kpt1")
            vist1 = colp.tile([128, 1], i32, tag="vist1", name="vist1")
            vist = colp.tile([NB, P * K], i32, tag="vist")
            negi_t = consts.tile([HL, HL], bf16, tag="negi")
            nc.scalar.dma_start(out=iota_t[:], in_=iota_d[:])
            nc.scalar.dma_start(out=negi_t[:], in_=negi_d[:])

            # ---- constants / big gen tiles
            ones_t = consts.tile([HL, 1], f32, tag="ones")
            accall = consts.tile([HL, NB], f32, tag="accall")
            gcol_all = consts.tile([128, NB * W], bf16, tag="gcol_all",
                                   name="gcol_all")
            grow_all = consts.tile([128, NB * W], bf16, tag="grow_all",
                                   name="grow_all")
            bd_all = consts.tile([128, NB * 4 * W], bf16, tag="bd_all",
                                 name="bd_all")
            nc.vector.memset(bd_all[:].bitcast(f32), 0.0)
            nc.gpsimd.memset(ones_t[:], 1.0 / B)

            # batched -trunc(kp*191), rounding-agnostic:
            # xf = round_any(t); trunc = xf - (xf > t); neg = -trunc
            def trunc_chain(kp_src, n, nm):
                tall = colp.tile([128, n], f32, tag=f"t_{nm}", name=f"t_{nm}")
                nall = colp.tile([128, n], f32, tag=f"n_{nm}", name=f"n_{nm}")
                xi = colp.tile([128, n], i32, tag=f"xi_{nm}", name=f"xi_{nm}")
                xf = colp.tile([128, n], f32, tag=f"xf_{nm}", name=f"xf_{nm}")
                nc.scalar.mul(tall[:], kp_src, float(W - 1))
                nc.vector.tensor_copy(xi[:], tall[:])
                nc.vector.tensor_copy(xf[:], xi[:])
                nc.vector.tensor_tensor(nall[:], xf[:], tall[:], Alu.is_gt)
                nc.vector.tensor_tensor(nall[:], nall[:], xf[:], Alu.subtract)
                return nall
            negd = trunc_chain(kpd[:], 2 * NB, "d")
            visfd = colp.tile([128, NB], f32, tag="visfd", name="visfd")
            nc.vector.tensor_copy(visfd[:], visd[:])

            # gcol: DVE add+mult, ScalarE Exp, densely per batch.
            for b in range(NB):
                dxc = colp.tile([128, W], f32, tag=f"gen_dxc{b % 2}",
                                name=f"gen_dxc{b % 2}")
                dxc2 = colp.tile([128, W], f32, tag=f"gen_dxc2{b % 2}",
                                 name=f"gen_dxc2{b % 2}")
                nc.vector.tensor_scalar_add(dxc[:], iota_t[:],
                                            negd[:, 2 * b + 1:2 * b + 2])
                nc.vector.tensor_tensor(dxc2[:], dxc[:], dxc[:], Alu.mult)
                nc.scalar.activation(gcol_all[:, b * W:(b + 1) * W], dxc2[:],
                                     Act.Exp, scale=-INV2S2)

            # scatter gcol into the block-diagonal tile: one SBUF->SBUF DMA
            # per k (contiguous 8-partition bands), split across both HWDGE
            # queues in k-order so bd group 0 lands first
            gsv = gcol_all[:].rearrange("p (b w) -> p b w", b=NB)
            bdv = bd_all[:].rearrange("p (b k w) -> p b k w", b=NB, k=4)
            for k in range(16):
                eng = nc.sync if k % 2 == 0 else nc.scalar
                eng.dma_start(
                    out=bdv[8 * k:8 * k + P, :, k % 4],
                    in_=gsv[8 * k:8 * k + P])
            # k16 kp/vis per batch behind the scatter on sync; their units
            # drain last
            for b in range(NB):
                nc.sync.dma_start(out=kpt1[32 * b:32 * b + P, 0:2],
                                  in_=kp_d[b, 16, :, 0:2])
                nc.sync.dma_start(out=vist1[32 * b:32 * b + P, 0:1],
                                  in_=vis_d[b, 16, :][:, None])
            nc.scalar.dma_start(out=vist[:], in_=vis_d.rearrange("b k p -> b (k p)"))

            # visibility normalizer: input-only, compute during the stream
            visf = colp.tile([NB, P * K], f32, tag="visf")
            nc.vector.tensor_copy(visf[:], vist[:])
            den = colp.tile([NB, 1], f32, tag="den")
            nc.vector.tensor_reduce(den[:], visf[:], axis=mybir.AxisListType.X,
                                    op=Alu.add)
            nc.vector.tensor_scalar_add(den[:], den[:], 1e-6)
            invd = colp.tile([NB, 1], f32, tag="invd")
            nc.vector.reciprocal(invd[:], den[:])

            # grow: DVE add+mult, ScalarE Exp, DVE vis gate
            for b in range(NB):
                dx = colp.tile([128, W], f32, tag=f"gen_dx{b % 2}",
                               name=f"gen_dx{b % 2}")
                dx2 = colp.tile([128, W], f32, tag=f"gen_dx2{b % 2}",
                                name=f"gen_dx2{b % 2}")
                nc.vector.tensor_scalar_add(dx[:], iota_t[:],
                                            negd[:, 2 * b:2 * b + 1])
                nc.vector.tensor_tensor(dx2[:], dx[:], dx[:], Alu.mult)
                gslice = grow_all[:, b * W:(b + 1) * W]
                nc.scalar.activation(gslice, dx2[:], Act.Exp, scale=-INV2S2)
                nc.vector.tensor_scalar_mul(gslice, gslice,
                                            visfd[:, b:b + 1])

            # k16 gen: partitions 32b+p
            negt1 = trunc_chain(kpt1[:], 2, "t1")
            visft1 = colp.tile([128, 1], f32, tag="visft1", name="visft1")
            nc.vector.tensor_copy(visft1[:], vist1[:])
            grow1 = consts.tile([128, W], bf16, tag="grow1")
            gcol1 = consts.tile([128, W], bf16, tag="gcol1")
            dc1 = colp.tile([128, W], f32, tag="dc1", name="dc1")
            nc.scalar.activation(dc1[:], iota_t[:], Act.Square,
                                 bias=negt1[:, 1:2])
            nc.scalar.activation(gcol1[:], dc1[:], Act.Exp, scale=-INV2S2)
            dr1 = colp.tile([128, W], f32, tag="dr1", name="dr1")
            dr2 = colp.tile([128, W], f32, tag="dr2", name="dr2")
            nc.vector.tensor_scalar_add(dr1[:], iota_t[:], negt1[:, 0:1])
            nc.vector.tensor_tensor(dr2[:], dr1[:], dr1[:], Alu.mult)
            nc.scalar.activation(grow1[:], dr2[:], Act.Exp, scale=-INV2S2)
            nc.vector.tensor_scalar_mul(grow1[:], grow1[:], visft1[:, 0:1])

            # ---------------- main loop: 20 pipelined units ----------------
            # One unit per (b, g): splat both halves, then either a PE
            # negI-subtract + ScalarE square straight from PSUM (one g per
            # batch, offloading DVE), or DVE subtract into a shared bf16
            # diff tile + one ScalarE square over both halves.
            accs_t = [consts.tile([HL, 2 * NG + 2], f32, tag=f"accs{b}",
                                  name=f"accs{b}") for b in range(NB)]
            for b in range(NB):
                for g in range(NG):
                    neg_unit = g == 1
                    dve_unit = g == 2
                    diff = None
                    if not neg_unit:
                        diff = scrp.tile([HL, 1536], bf16, tag="diff",
                                         name="diff")
                    for half in range(2):
                        pr = (plo_t if half == 0 else phi_t)[b]
                        ps = psump.tile([HL, 1024], f32, tag="ps", name="ps")
                        st = grow_all[32 * g:32 * g + 32,
                                      b * W + HL * half:b * W + HL * half + HL]
                        bdt = bd_all[32 * g:32 * g + 32,
                                     b * 4 * W:(b + 1) * 4 * W]
                        nc.tensor.matmul(ps[:, 0:384], st, bdt[:, 0:384],
                                         start=True, stop=not neg_unit,
                                         tile_position=(32 * g, 0))
                        nc.tensor.matmul(ps[:, 512:896], st, bdt[:, 384:768],
                                         start=True, stop=not neg_unit,
                                         tile_position=(32 * g, 0))
                        psv = ps[:].rearrange("p (a c) -> p a c", c=512)[:, :, 0:384]
                        if neg_unit:
                            nc.tensor.matmul(ps[:, 0:384], negi_t[:],
                                             pr[:, g * 768:g * 768 + 384],
                                             start=False, stop=True)
                            nc.tensor.matmul(ps[:, 512:896], negi_t[:],
                                             pr[:, g * 768 + 384:(g + 1) * 768],
                                             start=False, stop=True)
                            sq = sqp.tile([HL, 1536], bf16, tag="sq", name="sq")
                            sqv = sq[:, 0:768].rearrange("p (a c) -> p a c", c=384)
                            nc.scalar.activation(
                                sqv, psv, Act.Square,
                                accum_out=accs_t[b][:, 2 * g + half:
                                                    2 * g + half + 1])
                        else:
                            prv = pr[:, g * 768:(g + 1) * 768].rearrange(
                                "p (a c) -> p a c", c=384)
                            dv = diff[:, 768 * half:768 * (half + 1)].rearrange(
                                "p (a c) -> p a c", c=384)
                            nc.vector.tensor_tensor(dv, psv, prv, Alu.subtract)
                    if not neg_unit:
                        sq = sqp.tile([HL, 1536], bf16, tag="sq", name="sq")
                        if dve_unit and b >= 2:
                            nc.vector.affine_mul_reduce(
                                out=sq[:], accum_out=accs_t[b][:, 2 * g:2 * g + 1],
                                in0=diff[:], in1=diff[:], scale=1.0, bias=0.0)
                        else:
                            nc.scalar.activation(sq[:], diff[:], Act.Square,
                                                 accum_out=accs_t[b][:, 2 * g:
                                                                     2 * g + 1])

            # leftover k = 16 for all batches, drained after the main units
            for b in range(NB):
                ps = psump.tile([HL, 1024], f32, tag="ps", name="ps")
                l1 = grow1[32 * b:32 * b + P, 0:HL]
                h1 = grow1[32 * b:32 * b + P, HL:H]
                gc1 = gcol1[32 * b:32 * b + P, :]
                nc.tensor.matmul(ps[:, 0:192], l1, gc1, start=True, stop=False,
                                 tile_position=(32 * b, 0))
                nc.tensor.matmul(ps[:, 512:704], h1, gc1, start=True, stop=False,
                                 tile_position=(32 * b, 0))
                nc.tensor.matmul(ps[:, 0:192], negi_t[:],
                                 plo_t[b][:, 16 * W:17 * W],
                                 start=False, stop=True)
                nc.tensor.matmul(ps[:, 512:704], negi_t[:],
                                 phi_t[b][:, 16 * W:17 * W],
                                 start=False, stop=True)
                lview = ps[:].rearrange("p (a c) -> p a c", c=512)[:, :, 0:192]
                sq = sqp.tile([HL, 1536], bf16, tag="sq", name="sq")
                lsview = sq[:, 0:384].rearrange("p (a c) -> p a c", c=192)
                nc.scalar.activation(lsview, lview, Act.Square,
                                     accum_out=accs_t[b][:, 2 * NG:2 * NG + 1])

            for b in range(NB):
                nc.vector.tensor_reduce(accall[:, b:b + 1],
                                        accs_t[b][:, 0:2 * NG + 1],
                                        axis=mybir.AxisListType.X, op=Alu.add)

            # ---------------- finalize ----------------
            ps2 = psump.tile([HL, 1024], f32, tag="ps", name="ps")
            nc.tensor.matmul(ps2[0:NB, 0:1], accall[:, 0:NB], ones_t[:],
                             start=True, stop=True)
            outt = colp.tile([NB, 1], f32, tag="outt")
            nc.vector.tensor_tensor(outt[:], ps2[0:NB, 0:1], invd[:], Alu.mult)
            nc.sync.dma_start(out=out_d[:], in_=outt[:])

    nc.compile()
    return nc


def get_nc():
    if "nc" not in _CACHE:
        _CACHE["nc"] = _build()
    return _CACHE["nc"]


def make_in_maps(pred_heatmaps, keypoints, visibilities):
    pred = np.ascontiguousarray(pred_heatmaps, dtype=np.float32)
    # device expects [.., K, P, ..] layout so (k p) merges to a contiguous stride
    kp = np.ascontiguousarray(
        np.asarray(keypoints, dtype=np.float32).transpose(0, 2, 1, 3))
    vis = np.ascontiguousarray(
        np.asarray(visibilities, dtype=np.int32).transpose(0, 2, 1))
    iota = np.broadcast_to(np.arange(W, dtype=np.float32), (128, W)).copy()
    import ml_dtypes
    negi = (-np.eye(HL)).astype(ml_dtypes.bfloat16)
    in_maps = []
    for c in range(NCORES):
        sl = slice(c * NB, (c + 1) * NB)
        in_maps.append({
            "pred": pred[sl],
            "kp": kp[sl],
            "vis": vis[sl],
            "iota": iota,
            "negi": negi,
        })
    return in_maps


def kernel(pred_heatmaps, keypoints, visibilities):
    from concourse.bass_utils import run_bass_kernel_spmd

    nc = get_nc()
    in_maps = make_in_maps(pred_heatmaps, keypoints, visibilities)
    res = run_bass_kernel_spmd(nc, in_maps, core_ids=list(range(NCORES)))
    total = np.float64(0.0)
    for c in range(NCORES):
        total += np.asarray(res.results[c]["out"], dtype=np.float64).sum()
    return np.float32(total)


# revision 27
# speedup vs baseline: 1.0211x; 1.0211x over previous
"""Trainium2 Bass kernel for nn_KeypointsLoss.

Math (per batch b):
    x[p,k] = trunc(kp[b,p,k,0] * (W-1)); y likewise from kp[...,1]
    g_row[p,k,h] = exp(-(h-x)^2/(2s^2)) * (vis>0);  g_col[p,k,w] = exp(-(w-y)^2/(2s^2))
    target[k] = sum_p outer(g_row, g_col)            # [H,W]
    per_sample = sum_k |pred[b,k] - target[k]|^2
    loss = sum_b per_sample / (sum(vis[b]) + 1e-6) / B

Strategy (8 cores, data-parallel over B=32 -> 4 batches/core):
  - pred streams in f32 over HWDGE (sync queue) as 8 [96, K*W] h-major
    tiles; kp/vis/k16 inputs ride the scalar HWDGE queue so nothing
    queues behind the bulk stream.
  - gcol is generated (ScalarE Square+Exp) directly into one [128, 3072]
    block-diagonal tile via strided-partition APs (partitions 32g+8j+p,
    fixed j per op) -- no staircase DMAs, no per-k copies.
  - Splat: per (b, g, half) two 384-col matmuls with tile_position=
    (32g, 0); the four 32-row groups run concurrently in the PE array.
  - Drain: DVE subtracts pred from the PSUM target (bf16 out), ScalarE
    squares+accumulates; 36 small units pipeline against the DMA stream.
  - Tiny matmul with a (1/32)-vector reduces partitions; visibility
    normalizer on-device; host sums the 8x4 partials.
"""

import sys
import numpy as np

sys.path.insert(0, "/opt/trn_rl_repo")

B, P, K, H, W = 32, 8, 17, 192, 192
SIGMA = 3.0
INV2S2 = 1.0 / (2.0 * SIGMA**2)
NCORES = 8
NB = B // NCORES          # batches per core
HL = 96                   # h split: [0:96) lo, [96:192) hi
KW = K * W                # 3264 free cols for pred tiles
NG = 4                    # full k-groups of 4 (k0..15); k=16 handled separately

_CACHE = {}


def _build():
    import concourse.bass as bass
    import concourse.bacc as bacc
    import concourse.tile as tile
    from concourse import mybir

    f32 = mybir.dt.float32
    bf16 = mybir.dt.bfloat16
    i32 = mybir.dt.int32
    Alu = mybir.AluOpType
    Act = mybir.ActivationFunctionType

    nc = bacc.Bacc("TRN2", target_bir_lowering=False, debug=False,
                   num_devices=NCORES)

    pred_d = nc.dram_tensor("pred", [NB, K, H, W], f32, kind="ExternalInput").ap()
    kp_d = nc.dram_tensor("kp", [NB, K, P, 2], f32, kind="ExternalInput").ap()
    vis_d = nc.dram_tensor("vis", [NB, K, P], i32, kind="ExternalInput").ap()
    iota_d = nc.dram_tensor("iota", [128, W], f32, kind="ExternalInput").ap()
    negi_d = nc.dram_tensor("negi", [HL, HL], bf16, kind="ExternalInput").ap()
    out_d = nc.dram_tensor("out", [NB, 1], f32, kind="ExternalOutput").ap()

    with tile.TileContext(nc) as tc:
        import contextlib
        with contextlib.ExitStack() as ctx:
            consts = ctx.enter_context(tc.tile_pool(name="consts", bufs=1))
            colp = ctx.enter_context(tc.tile_pool(name="cols", bufs=1))
            predp = ctx.enter_context(tc.tile_pool(name="pred", bufs=1))
            scrp = ctx.enter_context(tc.tile_pool(name="scr", bufs=4))
            sqp = ctx.enter_context(tc.tile_pool(name="sq", bufs=2))
            psump = ctx.enter_context(tc.tile_pool(name="psum", bufs=4, space="PSUM"))

            # ---- bulk pred stream: 8 f32 DMAs on the sync HWDGE queue.
            # kp/vis ride first (they gate all generation work).
            kpd = colp.tile([128, 2 * NB], f32, tag="kpd", name="kpd")
            visd = colp.tile([128, NB], i32, tag="visd", name="visd")
            nc.sync.dma_start(
                out=kpd[:].rearrange("p (b t) -> p b t", t=2),
                in_=kp_d.rearrange("b k p t -> (k p) b t")[0:128])
            nc.sync.dma_start(
                out=visd[:],
                in_=vis_d.rearrange("b k p -> (k p) b")[0:128])

            # pred streams via SWDGE with f32->bf16 cast (HWDGE chokes on the
            # 1632-descriptor pattern; Q7 generation keeps up and halves the
            # SBUF write traffic)
            plo_t, phi_t = [], []
            for b in range(NB):
                psrc = pred_d[b].rearrange("k h w -> h k w")
                plo = predp.tile([HL, KW], bf16, tag=f"plo{b}", name=f"plo{b}")
                phi = predp.tile([HL, KW], bf16, tag=f"phi{b}", name=f"phi{b}")
                nc.gpsimd.dma_start(
                    out=plo[:].rearrange("p (k w) -> p k w", w=W),
                    in_=psrc[0:HL])
                nc.gpsimd.dma_start(
                    out=phi[:].rearrange("p (k w) -> p k w", w=W),
                    in_=psrc[HL:H])
                plo_t.append(plo)
                phi_t.append(phi)

            # ---- small inputs on the scalar HWDGE queue
            iota_t = consts.tile([128, W], f32, tag="iota")
            kpt1 = colp.tile([128, 2], f32, tag="kpt1", name="kpt1")
            vist1 = colp.tile([128, 1], i32, tag="vist1", name="vist1")
            vist = colp.tile([NB, P * K], i32, tag="vist")
            negi_t = consts.tile([HL, HL], bf16, tag="negi")
            nc.scalar.dma_start(out=iota_t[:], in_=iota_d[:])
            nc.scalar.dma_start(out=negi_t[:], in_=negi_d[:])

            # ---- constants / big gen tiles
            ones_t = consts.tile([HL, 1], f32, tag="ones")
            accall = consts.tile([HL, NB], f32, tag="accall")
            gcol_all = consts.tile([128, NB * W], bf16, tag="gcol_all",
                                   name="gcol_all")
            grow_all = consts.tile([128, NB * W], bf16, tag="grow_all",
                                   name="grow_all")
            bd_all = consts.tile([128, NB * 4 * W], bf16, tag="bd_all",
                                 name="bd_all")
            nc.vector.memset(bd_all[:].bitcast(f32), 0.0)
            nc.gpsimd.memset(ones_t[:], 1.0 / B)

            # batched -trunc(kp*191), rounding-agnostic:
            # xf = round_any(t); trunc = xf - (xf > t); neg = -trunc
            def trunc_chain(kp_src, n, nm):
                tall = colp.tile([128, n], f32, tag=f"t_{nm}", name=f"t_{nm}")
                nall = colp.tile([128, n], f32, tag=f"n_{nm}", name=f"n_{nm}")
                xi = colp.tile([128, n], i32, tag=f"xi_{nm}", name=f"xi_{nm}")
                xf = colp.tile([128, n], f32, tag=f"xf_{nm}", name=f"xf_{nm}")
                nc.scalar.mul(tall[:], kp_src, float(W - 1))
                nc.vector.tensor_copy(xi[:], tall[:])
                nc.vector.tensor_copy(xf[:], xi[:])
                nc.vector.tensor_tensor(nall[:], xf[:], tall[:], Alu.is_gt)
                nc.vector.tensor_tensor(nall[:], nall[:], xf[:], Alu.subtract)
                return nall
            negd = trunc_chain(kpd[:], 2 * NB, "d")
            visfd = colp.tile([128, NB], f32, tag="visfd", name="visfd")
            nc.vector.tensor_copy(visfd[:], visd[:])

            # gcol: DVE add+mult, ScalarE Exp, densely per batch.
            for b in range(NB):
                dxc = colp.tile([128, W], f32, tag=f"gen_dxc{b % 2}",
                                name=f"gen_dxc{b % 2}")
                dxc2 = colp.tile([128, W], f32, tag=f"gen_dxc2{b % 2}",
                                 name=f"gen_dxc2{b % 2}")
                nc.vector.tensor_scalar_add(dxc[:], iota_t[:],
                                            negd[:, 2 * b + 1:2 * b + 2])
                nc.vector.tensor_tensor(dxc2[:], dxc[:], dxc[:], Alu.mult)
                nc.scalar.activation(gcol_all[:, b * W:(b + 1) * W], dxc2[:],
                                     Act.Exp, scale=-INV2S2)

            # scatter gcol into the block-diagonal tile: one SBUF->SBUF DMA
            # per k (contiguous 8-partition bands), split across both HWDGE
            # queues in k-order so bd group 0 lands first
            gsv = gcol_all[:].rearrange("p (b w) -> p b w", b=NB)
            bdv = bd_all[:].rearrange("p (b k w) -> p b k w", b=NB, k=4)
            for k in range(16):
                eng = nc.sync if k % 2 == 0 else nc.scalar
                eng.dma_start(
                    out=bdv[8 * k:8 * k + P, :, k % 4],
                    in_=gsv[8 * k:8 * k + P])
            # k16 kp/vis per batch behind the scatter on sync; their units
            # drain last
            for b in range(NB):
                nc.sync.dma_start(out=kpt1[32 * b:32 * b + P, 0:2],
                                  in_=kp_d[b, 16, :, 0:2])
                nc.sync.dma_start(out=vist1[32 * b:32 * b + P, 0:1],
                                  in_=vis_d[b, 16, :][:, None])
            nc.scalar.dma_start(out=vist[:], in_=vis_d.rearrange("b k p -> b (k p)"))

            # visibility normalizer: input-only, compute during the stream
            visf = colp.tile([NB, P * K], f32, tag="visf")
            nc.vector.tensor_copy(visf[:], vist[:])
            den = colp.tile([NB, 1], f32, tag="den")
            nc.vector.tensor_reduce(den[:], visf[:], axis=mybir.AxisListType.X,
                                    op=Alu.add)
            nc.vector.tensor_scalar_add(den[:], den[:], 1e-6)
            invd = colp.tile([NB, 1], f32, tag="invd")
            nc.vector.reciprocal(invd[:], den[:])

            # grow: DVE add+mult, ScalarE Exp, DVE vis gate
            for b in range(NB):
                dx = colp.tile([128, W], f32, tag=f"gen_dx{b % 2}",
                               name=f"gen_dx{b % 2}")
                dx2 = colp.tile([128, W], f32, tag=f"gen_dx2{b % 2}",
                                name=f"gen_dx2{b % 2}")
                nc.vector.tensor_scalar_add(dx[:], iota_t[:],
                                            negd[:, 2 * b:2 * b + 1])
                nc.vector.tensor_tensor(dx2[:], dx[:], dx[:], Alu.mult)
                gslice = grow_all[:, b * W:(b + 1) * W]
                nc.scalar.activation(gslice, dx2[:], Act.Exp, scale=-INV2S2)
                nc.vector.tensor_scalar_mul(gslice, gslice,
                                            visfd[:, b:b + 1])

            # k16 gen: partitions 32b+p
            negt1 = trunc_chain(kpt1[:], 2, "t1")
            visft1 = colp.tile([128, 1], f32, tag="visft1", name="visft1")
            nc.vector.tensor_copy(visft1[:], vist1[:])
            grow1 = consts.tile([128, W], bf16, tag="grow1")
            gcol1 = consts.tile([128, W], bf16, tag="gcol1")
            dc1 = colp.tile([128, W], f32, tag="dc1", name="dc1")
            nc.scalar.activation(dc1[:], iota_t[:], Act.Square,
                                 bias=negt1[:, 1:2])
            nc.scalar.activation(gcol1[:], dc1[:], Act.Exp, scale=-INV2S2)
            dr1 = colp.tile([128, W], f32, tag="dr1", name="dr1")
            dr2 = colp.tile([128, W], f32, tag="dr2", name="dr2")
            nc.vector.tensor_scalar_add(dr1[:], iota_t[:], negt1[:, 0:1])
            nc.vector.tensor_tensor(dr2[:], dr1[:], dr1[:], Alu.mult)
            nc.scalar.activation(grow1[:], dr2[:], Act.Exp, scale=-INV2S2)
            nc.vector.tensor_scalar_mul(grow1[:], grow1[:], visft1[:, 0:1])

            # ---------------- main loop: 20 pipelined units ----------------
            # One unit per (b, g): splat both halves, then either a PE
            # negI-subtract + ScalarE square straight from PSUM (one g per
            # batch, offloading DVE), or DVE subtract into a shared bf16
            # diff tile + one ScalarE square over both halves.
            accs_t = [consts.tile([HL, 2 * NG + 2], f32, tag=f"accs{b}",
                                  name=f"accs{b}") for b in range(NB)]
            for b in range(NB):
                for g in range(NG):
                    neg_unit = g == 1
                    dve_unit = g == 2
                    diff = None
                    if not neg_unit:
                        diff = scrp.tile([HL, 1536], bf16, tag="diff",
                                         name="diff")
                    for half in range(2):
                        pr = (plo_t if half == 0 else phi_t)[b]
                        ps = psump.tile([HL, 1024], f32, tag="ps", name="ps")
                        st = grow_all[32 * g:32 * g + 32,
                                      b * W + HL * half:b * W + HL * half + HL]
                        bdt = bd_all[32 * g:32 * g + 32,
                                     b * 4 * W:(b + 1) * 4 * W]
                        nc.tensor.matmul(ps[:, 0:384], st, bdt[:, 0:384],
                                         start=True, stop=not neg_unit,
                                         tile_position=(32 * g, 0))
                        nc.tensor.matmul(ps[:, 512:896], st, bdt[:, 384:768],
                                         start=True, stop=not neg_unit,
                                         tile_position=(32 * g, 0))
                        psv = ps[:].rearrange("p (a c) -> p a c", c=512)[:, :, 0:384]
                        if neg_unit:
                            nc.tensor.matmul(ps[:, 0:384], negi_t[:],
                                             pr[:, g * 768:g * 768 + 384],
                                             start=False, stop=True)
                            nc.tensor.matmul(ps[:, 512:896], negi_t[:],
                                             pr[:, g * 768 + 384:(g + 1) * 768],
                                             start=False, stop=True)
                            sq = sqp.tile([HL, 1536], bf16, tag="sq", name="sq")
                            sqv = sq[:, 0:768].rearrange("p (a c) -> p a c", c=384)
                            nc.scalar.activation(
                                sqv, psv, Act.Square,
                                accum_out=accs_t[b][:, 2 * g + half:
                                                    2 * g + half + 1])
                        else:
                            prv = pr[:, g * 768:(g + 1) * 768].rearrange(
                                "p (a c) -> p a c", c=384)
                            dv = diff[:, 768 * half:768 * (half + 1)].rearrange(
                                "p (a c) -> p a c", c=384)
                            nc.vector.tensor_tensor(dv, psv, prv, Alu.subtract)
                    if not neg_unit:
                        sq = sqp.tile([HL, 1536], bf16, tag="sq", name="sq")
                        if dve_unit:
                            nc.vector.affine_mul_reduce(
                                out=sq[:], accum_out=accs_t[b][:, 2 * g:2 * g + 1],
                                in0=diff[:], in1=diff[:], scale=1.0, bias=0.0)
                        else:
                            nc.scalar.activation(sq[:], diff[:], Act.Square,
                                                 accum_out=accs_t[b][:, 2 * g:
                                                                     2 * g + 1])

            # leftover k = 16 for all batches, drained after the main units
            for b in range(NB):
                ps = psump.tile([HL, 1024], f32, tag="ps", name="ps")
                l1 = grow1[32 * b:32 * b + P, 0:HL]
                h1 = grow1[32 * b:32 * b + P, HL:H]
                gc1 = gcol1[32 * b:32 * b + P, :]
                nc.tensor.matmul(ps[:, 0:192], l1, gc1, start=True, stop=False,
                                 tile_position=(32 * b, 0))
                nc.tensor.matmul(ps[:, 512:704], h1, gc1, start=True, stop=False,
                                 tile_position=(32 * b, 0))
                nc.tensor.matmul(ps[:, 0:192], negi_t[:],
                                 plo_t[b][:, 16 * W:17 * W],
                                 start=False, stop=True)
                nc.tensor.matmul(ps[:, 512:704], negi_t[:],
                                 phi_t[b][:, 16 * W:17 * W],
                                 start=False, stop=True)
                lview = ps[:].rearrange("p (a c) -> p a c", c=512)[:, :, 0:192]
                sq = sqp.tile([HL, 1536], bf16, tag="sq", name="sq")
                lsview = sq[:, 0:384].rearrange("p (a c) -> p a c", c=192)
                nc.scalar.activation(lsview, lview, Act.Square,
                                     accum_out=accs_t[b][:, 2 * NG:2 * NG + 1])

            for b in range(NB):
                nc.vector.tensor_reduce(accall[:, b:b + 1],
                                        accs_t[b][:, 0:2 * NG + 1],
                                        axis=mybir.AxisListType.X, op=Alu.add)

            # ---------------- finalize ----------------
            ps2 = psump.tile([HL, 1024], f32, tag="ps", name="ps")
            nc.tensor.matmul(ps2[0:NB, 0:1], accall[:, 0:NB], ones_t[:],
                             start=True, stop=True)
            outt = colp.tile([NB, 1], f32, tag="outt")
            nc.vector.tensor_tensor(outt[:], ps2[0:NB, 0:1], invd[:], Alu.mult)
            nc.sync.dma_start(out=out_d[:], in_=outt[:])

    nc.compile()
    return nc


def get_nc():
    if "nc" not in _CACHE:
        _CACHE["nc"] = _build()
    return _CACHE["nc"]


def make_in_maps(pred_heatmaps, keypoints, visibilities):
    pred = np.ascontiguousarray(pred_heatmaps, dtype=np.float32)
    # device expects [.., K, P, ..] layout so (k p) merges to a contiguous stride
    kp = np.ascontiguousarray(
        np.asarray(keypoints, dtype=np.float32).transpose(0, 2, 1, 3))
    vis = np.ascontiguousarray(
        np.asarray(visibilities, dtype=np.int32).transpose(0, 2, 1))
    iota = np.broadcast_to(np.arange(W, dtype=np.float32), (128, W)).copy()
    import ml_dtypes
    negi = (-np.eye(HL)).astype(ml_dtypes.bfloat16)
    in_maps = []
    for c in range(NCORES):
        sl = slice(c * NB, (c + 1) * NB)
        in_maps.append({
            "pred": pred[sl],
            "kp": kp[sl],
            "vis": vis[sl],
            "iota": iota,
            "negi": negi,
        })
    return in_maps


def kernel(pred_heatmaps, keypoints, visibilities):
    from concourse.bass_utils import run_bass_kernel_spmd

    nc = get_nc()
    in_maps = make_in_maps(pred_heatmaps, keypoints, visibilities)
    res = run_bass_kernel_spmd(nc, in_maps, core_ids=list(range(NCORES)))
    total = np.float64(0.0)
    for c in range(NCORES):
        total += np.asarray(res.results[c]["out"], dtype=np.float64).sum()
    return np.float32(total)
